# revision 6
# baseline (speedup 1.0000x reference)
"""Multi-head self-attention (B=2, T=2048, C=1024, H=16) on 8 trn2 cores.

Sharding: core c -> batch b = c//4, heads 4*(c%4) .. 4*(c%4)+3.
Each core: QKV projection for its 4 heads, causal attention in S^T layout
(keys on partitions), partial output projection over its heads' rows of Wo.
Host sums the 4 partials per batch element and adds bo.

Structure (v2):
- softmax division is per-slab: reciprocal of the PV denominator row,
  gpsimd partition-broadcast, and the divide fused into the PSUM->SBUF
  copy of the PV output. Division for slab s is emitted inside slab s+1
  so it never blocks the attention pipeline.
- output projection is interleaved into the attention loop (one token
  block at a time, as soon as both head-pairs of its slab are divided),
  sharing the S-matmul PSUM ring; its DMA drains during attention.
- ROWTILE: S matmuls use PE row tiling (two K=64 heads run concurrently
  in row groups 0-1 / 2-3) instead of zero-padding K to 128.

All matmuls run in float32r (fp32 with 12-bit mantissa, full PE rate).
"""
import sys

sys.path.insert(0, "/opt/trn_rl_repo")

import numpy as np

B, T, C, H = 2, 2048, 1024, 16
HD = C // H            # 64
NCORES = 8
HPC = H // (NCORES // B)   # heads per core = 4
QB = 128               # q block (columns of S^T)
KB = 128               # k chunk (partitions of S^T)
NJ = T // KB           # 16
NI = T // QB           # 16
SLAB = 512             # q columns processed per attention pass
NSLAB = T // SLAB      # 4
BPS = SLAB // QB       # q blocks per slab = 4
CI = C // 128          # 8 contraction chunks for projections
SCALE = HD ** -0.5
ROWTILE = False

_cache = {}


def _round_fp32r(x: np.ndarray) -> np.ndarray:
    u = np.ascontiguousarray(x, dtype=np.float32).view(np.uint32)
    r = (u + 0x7FF + ((u >> 12) & 1)) & np.uint32(0xFFFFF000)
    return r.view(np.float32)


def _build_plan(mask_bool: np.ndarray):
    """mask_bool: [T, T] (q, k). Returns per (j, i) block types and tiles.

    type 0 = all valid (no mask work), 1 = all masked (skip), 2 = mixed.
    Tiles are stored transposed to match S^T ([k_local, q_local])."""
    btype = np.zeros((NJ, NI), dtype=np.int32)
    tidx = np.full((NJ, NI), -1, dtype=np.int32)
    tiles = []
    tile_map = {}
    for j in range(NJ):
        for i in range(NI):
            sub = mask_bool[i * QB:(i + 1) * QB, j * KB:(j + 1) * KB]
            if sub.all():
                btype[j, i] = 0
            elif not sub.any():
                btype[j, i] = 1
            else:
                btype[j, i] = 2
                key = sub.tobytes()
                if key not in tile_map:
                    tile_map[key] = len(tiles)
                    tiles.append(sub.T.astype(np.float32))
                tidx[j, i] = tile_map[key]
    if not tiles:
        tiles.append(np.ones((KB, QB), dtype=np.float32))
    return btype, tidx, np.stack(tiles)


def _build_program(btype, tidx, n_tiles, apply_qk_bias, apply_v_bias):
    import concourse.bass as bass
    import concourse.tile as tile
    import concourse.mybir as mybir
    from concourse import bacc

    F32 = mybir.dt.float32
    F32R = mybir.dt.float32r
    AF = mybir.ActivationFunctionType
    MULT = mybir.AluOpType.mult

    nc = bacc.Bacc("TRN2", target_bir_lowering=False, debug=False)
    xt_d = nc.dram_tensor("xt", [C, T], F32R, kind="ExternalInput").ap()
    wqk_d = nc.dram_tensor("wqk", [C, 4 * 128], F32R, kind="ExternalInput").ap()
    wv_d = nc.dram_tensor("wv", [C, HPC * HD], F32R, kind="ExternalInput").ap()
    wo_d = nc.dram_tensor("wo", [HPC * HD, C], F32R, kind="ExternalInput").ap()
    mask_d = nc.dram_tensor("masks", [n_tiles, KB, QB], F32,
                            kind="ExternalInput").ap()
    bqk_d = nc.dram_tensor("bqk", [128, 4], F32, kind="ExternalInput").ap()
    bv_d = nc.dram_tensor("bv", [128, 2], F32, kind="ExternalInput").ap()
    out_d = nc.dram_tensor("out", [T, C], F32, kind="ExternalOutput").ap()

    with tile.TileContext(nc) as tc:
        with tc.tile_pool(name="weights", bufs=1) as wpool, \
             tc.tile_pool(name="acts", bufs=1) as apool:
            # ---- resident SBUF tensors ----
            wo = wpool.tile([128, 2, C], F32R)
            masks = wpool.tile([128, n_tiles * QB], F32)
            bqk = wpool.tile([128, 4], F32)
            bv = wpool.tile([128, 2], F32)
            # q tiles hold (q_hA | q_hB) on partitions 0-63 / 64-127.
            qp = [apool.tile([128, T], F32R, tag=f"qp{i}", name=f"qp{i}")
                  for i in range(2)]
            if ROWTILE:
                # k in the same (hA | hB) layout; S matmuls row-tile the
                # two K=64 heads into PE row groups 0-1 / 2-3.
                kp = [apool.tile([128, T], F32R, tag=f"kp{i}", name=f"kp{i}")
                      for i in range(2)]
            else:
                # k zero-padded per head so S matmuls present K=128.
                kz = [apool.tile([128, T], F32R, tag=f"kz{i}", name=f"kz{i}")
                      for i in range(4)]
            vaug = apool.tile([128, NJ, HPC * (HD + 1)], F32R)

            # ---- QKV projection ----
            # q/k in transposed layout: psum[c_out_pair, t] = W^T x^T
            with tc.tile_pool(name="xtp", bufs=1) as xtp, \
                 tc.tile_pool(name="pproj", bufs=4, space="PSUM") as pp:
                xt = xtp.tile([128, CI, T], F32R)      # x^T, c_in chunked
                wqk = xtp.tile([128, CI, 512], F32R)
                wv = xtp.tile([128, CI, HPC * HD], F32R)
                qt4 = T // 4
                # DMA order: first-needed first. co=0 walks the four
                # T-quarters in order, so interleave wqk with quarter 0,
                # then stream the later quarters / wv / wo behind it.
                for ci in range(CI):
                    nc.sync.dma_start(wqk[:, ci, :],
                                      wqk_d[ci * 128:(ci + 1) * 128, :])
                    nc.sync.dma_start(xt[:, ci, 0:qt4],
                                      xt_d[ci * 128:(ci + 1) * 128, 0:qt4])
                for ci in range(CI):
                    nc.sync.dma_start(xt[:, ci, qt4:2 * qt4],
                                      xt_d[ci * 128:(ci + 1) * 128,
                                           qt4:2 * qt4])
                for ci in range(CI):
                    nc.sync.dma_start(xt[:, ci, 2 * qt4:3 * qt4],
                                      xt_d[ci * 128:(ci + 1) * 128,
                                           2 * qt4:3 * qt4])
                for ci in range(CI):
                    nc.sync.dma_start(wv[:, ci, :],
                                      wv_d[ci * 128:(ci + 1) * 128, :])
                    nc.sync.dma_start(xt[:, ci, 3 * qt4:T],
                                      xt_d[ci * 128:(ci + 1) * 128, 3 * qt4:T])
                nc.sync.dma_start(wo[:, 0, :], wo_d[0:128, :])
                nc.sync.dma_start(wo[:, 1, :], wo_d[128:256, :])
                for t in range(n_tiles):
                    nc.sync.dma_start(masks[:, t * QB:(t + 1) * QB], mask_d[t])
                if apply_qk_bias:
                    nc.sync.dma_start(bqk[:], bqk_d)
                if apply_v_bias:
                    nc.sync.dma_start(bv[:], bv_d)
                if not ROWTILE:
                    for p in range(2):
                        nc.vector.tensor_copy(
                            kz[2 * p][64:128, :],
                            nc.const_aps.tensor(0.0, (64, T)))
                        nc.vector.tensor_copy(
                            kz[2 * p + 1][0:64, :],
                            nc.const_aps.tensor(0.0, (64, T)))
                va = vaug[:].rearrange("p j (h d) -> p j h d", h=HPC)
                nc.vector.tensor_copy(
                    va[:, :, :, HD:HD + 1],
                    nc.const_aps.tensor(1.0, (128, NJ, HPC, 1)))
                for co in (0, 1, "v", 2, 3):
                    if co == "v":
                        for tj in range(NJ):
                            psv = pp.tile([128, HPC * HD], F32, tag="pv",
                                          name="psv")
                            for ci in range(CI):
                                nc.tensor.matmul(
                                    psv[:],
                                    xt[:, ci, tj * 128:(tj + 1) * 128],
                                    wv[:, ci, :],
                                    start=(ci == 0), stop=(ci == CI - 1))
                            nc.vector.tensor_copy(
                                va[:, tj, :, 0:HD],
                                psv[:].rearrange("p (h d) -> p h d", h=HPC))
                        continue
                    pair, is_k = co // 2, co % 2
                    for ts in range(T // 512):
                        sl = slice(ts * 512, (ts + 1) * 512)
                        ps = pp.tile([128, 512], F32, tag="pqk")
                        for ci in range(CI):
                            nc.tensor.matmul(
                                ps[:],
                                wqk[:, ci, co * 128:(co + 1) * 128],
                                xt[:, ci, sl],
                                start=(ci == 0), stop=(ci == CI - 1))
                        if is_k:
                            if ROWTILE:
                                dsts = [(kp[pair][:, sl], ps[:],
                                         bqk[:, co:co + 1])]
                            else:
                                dsts = [(kz[2 * pair][0:64, sl], ps[0:64, :],
                                         bqk[0:64, co:co + 1]),
                                        (kz[2 * pair + 1][64:128, sl],
                                         ps[64:128, :],
                                         bqk[64:128, co:co + 1])]
                        else:
                            dsts = [(qp[pair][:, sl], ps[:],
                                     bqk[:, co:co + 1])]
                        for dst_ap, src_ap, b_ap in dsts:
                            if apply_qk_bias:
                                nc.scalar.activation(dst_ap, src_ap,
                                                     AF.Identity, bias=b_ap,
                                                     scale=1.0)
                            else:
                                nc.vector.tensor_copy(dst_ap, src_ap)

            # ---- attention (S^T layout) + interleaved output projection ----
            with tc.tile_pool(name="attnp", bufs=1) as attnp:
              attn = [attnp.tile([128, T], F32R, tag=f"attn{p}",
                                 name=f"attn{p}") for p in range(2)]
              with tc.tile_pool(name="psattn", bufs=1, space="PSUM") as sp, \
                   tc.tile_pool(name="psout", bufs=1, space="PSUM") as op, \
                   tc.tile_pool(name="ptp", bufs=5) as ptp, \
                   tc.tile_pool(name="divp", bufs=2) as divp, \
                   tc.tile_pool(name="osb", bufs=2) as osb:
                  pending_div = []   # (pair, [out_ps hl0, hl1], s)
                  pending_proj = []  # token-block indices ready to project

                  def emit_division(pair, tiles_hl, s):
                      # per-slab softmax divide: reciprocal of the PV
                      # denominator row, broadcast across partitions, and
                      # multiply fused into the PSUM->SBUF copy.
                      recs = []
                      for hl in range(2):
                          sums = divp.tile([1, SLAB], F32, tag="sums",
                                           name="sums")
                          nc.vector.tensor_copy(sums[:],
                                                tiles_hl[hl][HD:HD + 1, :])
                          rec1 = divp.tile([1, SLAB], F32, tag="rec1",
                                           name="rec1")
                          nc.vector.reciprocal_approx_fast(rec1[:], sums[:])
                          rec128 = divp.tile([128, SLAB], F32,
                                             tag=f"rec128_{hl}",
                                             name="rec128")
                          nc.gpsimd.partition_broadcast(rec128[:], rec1[:])
                          recs.append(rec128)
                      for hl in range(2):
                          dst = attn[pair][64 * hl:64 * hl + 64,
                                           s * SLAB:(s + 1) * SLAB]
                          nc.vector.tensor_tensor(
                              out=dst, in0=tiles_hl[hl][0:HD, :],
                              in1=recs[hl][64 * hl:64 * hl + 64, :], op=MULT)
                          if apply_v_bias:
                              nc.vector.tensor_scalar(
                                  out=dst, in0=dst,
                                  scalar1=bv[64 * hl:64 * hl + 64,
                                             pair:pair + 1],
                                  scalar2=None, op0=mybir.AluOpType.add)
                      if pair == 1:
                          pending_proj.extend(range(BPS * s, BPS * (s + 1)))

                  def emit_outproj(ts):
                      # one 128-token block of the output projection; shares
                      # the S-matmul PSUM ring ("sst").
                      ps = sp.tile([128, 2, SLAB], F32, tag="sst",
                                   name="psop", bufs=2)
                      for h in range(2):
                          for pair in range(2):
                              nc.tensor.matmul(
                                  ps[:, h, :],
                                  attn[pair][:, ts * 128:(ts + 1) * 128],
                                  wo[:, pair, h * 512:(h + 1) * 512],
                                  start=(pair == 0), stop=(pair == 1))
                      ot = osb.tile([128, C], F32, tag="ot", name="ot")
                      nc.vector.tensor_copy(
                          ot[:].rearrange("p (h q) -> p h q", h=2), ps[:])
                      nc.sync.dma_start(out_d[ts * 128:(ts + 1) * 128, :],
                                        ot[:])

                  for pair in range(2):
                      q_t = qp[pair]
                      for s in range(NSLAB):
                          i_lo, i_hi = s * BPS, (s + 1) * BPS
                          chunks = []
                          for j in range(NJ):
                              live = [i for i in range(i_lo, i_hi)
                                      if btype[j, i] != 1]
                              if live:
                                  chunks.append((j, min(live), max(live)))
                          out_ps = [op.tile([HD + 1, SLAB], F32,
                                            tag=f"outps{_hl}",
                                            name=f"outps{_hl}", bufs=2)
                                    for _hl in range(2)]
                          written = np.zeros(BPS, dtype=bool)
                          for cn, (j, i0, i1) in enumerate(chunks):
                              n_cols = (i1 - i0 + 1) * QB
                              r0 = i0 - i_lo
                              # S^T for both heads into the two banks of one
                              # psum tile; one exp covers both
                              sps = sp.tile([128, 2, SLAB], F32,
                                            tag="sst", name="sst", bufs=2)
                              if ROWTILE:
                                  for hl in range(2):
                                      nc.tensor.matmul(
                                          sps[:, hl, 0:n_cols],
                                          kp[pair][64 * hl:64 * hl + 64,
                                                   j * KB:(j + 1) * KB],
                                          q_t[64 * hl:64 * hl + 64,
                                              i0 * QB:i0 * QB + n_cols],
                                          start=True, stop=True)
                              else:
                                  for hl in range(2):
                                      nc.tensor.matmul(
                                          sps[:, hl, 0:n_cols],
                                          kz[2 * pair + hl][:,
                                                            j * KB:(j + 1) * KB],
                                          q_t[:, i0 * QB:i0 * QB + n_cols],
                                          start=True, stop=True)
                              pt = ptp.tile([128, 2, SLAB], F32R, tag="pt",
                                            name="pt")
                              nc.scalar.activation(pt[:, :, 0:n_cols],
                                                   sps[:, :, 0:n_cols],
                                                   AF.Exp, scale=SCALE)
                              for i in range(i0, i1 + 1):
                                  rel = (i - i0) * QB
                                  if btype[j, i] == 2:
                                      ti = tidx[j, i]
                                      m2 = masks[:, ti * QB:(ti + 1) * QB] \
                                          .unsqueeze(1).broadcast_to(
                                              [128, 2, QB])
                                      nc.gpsimd.tensor_tensor(
                                          out=pt[:, :, rel:rel + QB],
                                          in0=pt[:, :, rel:rel + QB],
                                          in1=m2, op=MULT)
                                  elif btype[j, i] == 1:
                                      nc.gpsimd.memset(pt[:, :, rel:rel + QB],
                                                       0.0)
                              # previous slab's division goes out right after
                              # this slab's first chunk is in flight
                              if cn == 1 and pending_div:
                                  emit_division(*pending_div.pop(0))
                              # outproj insertions go in pairs so the shared
                              # "sst" psum ring keeps its even double-buffer
                              # phase for the S-matmul pipeline
                              if cn >= 2 and cn % 2 == 0 and pending_proj:
                                  emit_outproj(pending_proj.pop(0))
                                  if pending_proj:
                                      emit_outproj(pending_proj.pop(0))
                              # PV accumulation (runs are <= 512 so no bank
                              # crossing; split only on first-write transitions)
                              segs = []
                              c = r0 * QB
                              end = (i1 - i_lo + 1) * QB
                              while c < end:
                                  st = written[c // QB]
                                  cc = c + QB
                                  while cc < end and written[cc // QB] == st:
                                      cc += QB
                                  segs.append((c, cc, not st))
                                  c = cc
                              last = cn == len(chunks) - 1
                              for hl in range(2):
                                  hh = 2 * pair + hl
                                  for (c0, c1, st_flag) in segs:
                                      nc.tensor.matmul(
                                          out_ps[hl][:, c0:c1],
                                          vaug[:, j, hh * (HD + 1):
                                               (hh + 1) * (HD + 1)],
                                          pt[:, hl, c0 - r0 * QB:c1 - r0 * QB],
                                          start=st_flag, stop=last,
                                          skip_group_check=True)
                              for rr in range(r0, i1 - i_lo + 1):
                                  written[rr] = True
                          pending_div.append((pair, out_ps, s))
                  while pending_div:
                      emit_division(*pending_div.pop(0))
                  while pending_proj:
                      emit_outproj(pending_proj.pop(0))

    nc.compile()
    return nc


def _get_program(mask_bool, apply_qk_bias, apply_v_bias):
    key = (mask_bool.tobytes(), apply_qk_bias, apply_v_bias, ROWTILE)
    if key not in _cache:
        btype, tidx, tiles = _build_plan(mask_bool)
        nc = _build_program(btype, tidx, len(tiles), apply_qk_bias,
                            apply_v_bias)
        _cache[key] = (nc, tiles)
    return _cache[key]


def kernel(x, attention_mask, Wqkv, bqkv, Wo, bo, _trace=False):
    from concourse.bass_utils import run_bass_kernel_spmd

    x = np.asarray(x, dtype=np.float32)
    mask_bool = np.asarray(attention_mask)[0, 0] != 0
    Wqkv = np.asarray(Wqkv, dtype=np.float32)
    bqkv = np.asarray(bqkv, dtype=np.float32)
    Wo = np.asarray(Wo, dtype=np.float32)
    bo = np.asarray(bo, dtype=np.float32)

    apply_qk_bias = bool(np.any(bqkv[:2 * C]))
    apply_v_bias = bool(np.any(bqkv[2 * C:]))
    nc, tiles = _get_program(mask_bool, apply_qk_bias, apply_v_bias)

    xts = [_round_fp32r(x[b].T) for b in range(B)]
    in_maps = []
    for c in range(NCORES):
        b, g = divmod(c, NCORES // B)
        hs = [HPC * g + i for i in range(HPC)]
        # wqk column chunks: [q_h0|q_h1, k_h0|k_h1, q_h2|q_h3, k_h2|k_h3]
        cols, bias_cols = [], []
        for pair in range(2):
            ha, hb = hs[2 * pair], hs[2 * pair + 1]
            for base in (0, C):  # q then k offset in Wqkv columns
                cols.append(Wqkv[:, base + ha * HD:base + (ha + 1) * HD])
                cols.append(Wqkv[:, base + hb * HD:base + (hb + 1) * HD])
                bias_cols.append(np.concatenate([
                    bqkv[base + ha * HD:base + (ha + 1) * HD],
                    bqkv[base + hb * HD:base + (hb + 1) * HD]]))
        wqk_c = _round_fp32r(np.concatenate(cols, axis=1))
        bqk_c = np.stack(bias_cols, axis=1).astype(np.float32)
        wv_c = _round_fp32r(np.concatenate(
            [Wqkv[:, 2 * C + h * HD:2 * C + (h + 1) * HD] for h in hs], axis=1))
        wo_c = _round_fp32r(np.concatenate(
            [Wo[h * HD:(h + 1) * HD, :] for h in hs], axis=0))
        bv_c = np.zeros((128, 2), dtype=np.float32)
        for pair in range(2):
            ha, hb = hs[2 * pair], hs[2 * pair + 1]
            bv_c[0:HD, pair] = bqkv[2 * C + ha * HD:2 * C + (ha + 1) * HD]
            bv_c[HD:128, pair] = bqkv[2 * C + hb * HD:2 * C + (hb + 1) * HD]
        in_maps.append({
            "xt": xts[b], "wqk": wqk_c, "wv": wv_c, "wo": wo_c,
            "masks": tiles, "bqk": bqk_c, "bv": bv_c,
        })

    kwargs = {}
    if _trace:
        kwargs = dict(trace=True, trace_cores=[0])
    res = run_bass_kernel_spmd(nc, in_maps, core_ids=list(range(NCORES)),
                               **kwargs)
    out = np.empty((B, T, C), dtype=np.float32)
    gpb = NCORES // B
    for b in range(B):
        acc = res.results[b * gpb]["out"].astype(np.float32)
        for g in range(1, gpb):
            acc = acc + res.results[b * gpb + g]["out"]
        out[b] = acc + bo
    if _trace:
        kernel._last_results = res
    return out


# revision 9
# speedup vs baseline: 1.3545x; 1.3545x over previous
"""Multi-head self-attention (B=2, T=2048, C=1024, H=16) on 8 trn2 cores.

Sharding: core c -> batch b = c//4, heads 4*(c%4) .. 4*(c%4)+3.
Each core: QKV projection for its 4 heads, causal attention in S^T layout
(keys on partitions), partial output projection over its heads' rows of Wo.
Host sums the 4 partials per batch element and adds bo.

Structure (v2):
- softmax division is per-slab: reciprocal of the PV denominator row,
  gpsimd partition-broadcast, and the divide fused into the PSUM->SBUF
  copy of the PV output. Division for slab s is emitted inside slab s+1
  so it never blocks the attention pipeline.
- output projection is interleaved into the attention loop (one token
  block at a time, as soon as both head-pairs of its slab are divided),
  sharing the S-matmul PSUM ring; its DMA drains during attention.
- ROWTILE: S matmuls use PE row tiling (two K=64 heads run concurrently
  in row groups 0-1 / 2-3) instead of zero-padding K to 128.

All matmuls run in float32r (fp32 with 12-bit mantissa, full PE rate).
"""
import sys

sys.path.insert(0, "/opt/trn_rl_repo")

import numpy as np

B, T, C, H = 2, 2048, 1024, 16
HD = C // H            # 64
NCORES = 8
HPC = H // (NCORES // B)   # heads per core = 4
QB = 128               # q block (columns of S^T)
KB = 128               # k chunk (partitions of S^T)
NJ = T // KB           # 16
NI = T // QB           # 16
SLAB = 512             # q columns processed per attention pass
NSLAB = T // SLAB      # 4
BPS = SLAB // QB       # q blocks per slab = 4
CI = C // 128          # 8 contraction chunks for projections
SCALE = HD ** -0.5
ROWTILE = False

_cache = {}


def _round_fp32r(x: np.ndarray) -> np.ndarray:
    u = np.ascontiguousarray(x, dtype=np.float32).view(np.uint32)
    r = (u + 0x7FF + ((u >> 12) & 1)) & np.uint32(0xFFFFF000)
    return r.view(np.float32)


def _build_plan(mask_bool: np.ndarray):
    """mask_bool: [T, T] (q, k). Returns per (j, i) block types and tiles.

    type 0 = all valid (no mask work), 1 = all masked (skip), 2 = mixed.
    Tiles are stored transposed to match S^T ([k_local, q_local])."""
    btype = np.zeros((NJ, NI), dtype=np.int32)
    tidx = np.full((NJ, NI), -1, dtype=np.int32)
    tiles = []
    tile_map = {}
    for j in range(NJ):
        for i in range(NI):
            sub = mask_bool[i * QB:(i + 1) * QB, j * KB:(j + 1) * KB]
            if sub.all():
                btype[j, i] = 0
            elif not sub.any():
                btype[j, i] = 1
            else:
                btype[j, i] = 2
                key = sub.tobytes()
                if key not in tile_map:
                    tile_map[key] = len(tiles)
                    tiles.append(sub.T.astype(np.float32))
                tidx[j, i] = tile_map[key]
    if not tiles:
        tiles.append(np.ones((KB, QB), dtype=np.float32))
    return btype, tidx, np.stack(tiles)


def _build_program(btype, tidx, n_tiles, apply_qk_bias, apply_v_bias):
    import concourse.bass as bass
    import concourse.tile as tile
    import concourse.mybir as mybir
    from concourse import bacc

    F32 = mybir.dt.float32
    F32R = mybir.dt.float32r
    AF = mybir.ActivationFunctionType
    MULT = mybir.AluOpType.mult

    nc = bacc.Bacc("TRN2", target_bir_lowering=False, debug=False)
    xt_d = nc.dram_tensor("xt", [C, T], F32R, kind="ExternalInput").ap()
    wqk_d = nc.dram_tensor("wqk", [C, 4 * 128], F32R, kind="ExternalInput").ap()
    wv_d = nc.dram_tensor("wv", [C, HPC * HD], F32R, kind="ExternalInput").ap()
    wo_d = nc.dram_tensor("wo", [HPC * HD, C], F32R, kind="ExternalInput").ap()
    mask_d = nc.dram_tensor("masks", [n_tiles, KB, QB], F32,
                            kind="ExternalInput").ap()
    bqk_d = nc.dram_tensor("bqk", [128, 4], F32, kind="ExternalInput").ap()
    bv_d = nc.dram_tensor("bv", [128, 2], F32, kind="ExternalInput").ap()
    out_d = nc.dram_tensor("out", [T, C], F32, kind="ExternalOutput").ap()

    with tile.TileContext(nc) as tc:
        with tc.tile_pool(name="weights", bufs=1) as wpool, \
             tc.tile_pool(name="acts", bufs=1) as apool:
            # ---- resident SBUF tensors ----
            wo = wpool.tile([128, 2, C], F32R)
            masks = wpool.tile([128, n_tiles * QB], F32)
            bqk = wpool.tile([128, 4], F32)
            bv = wpool.tile([128, 2], F32)
            # q tiles hold (q_hA | q_hB) on partitions 0-63 / 64-127.
            qp = [apool.tile([128, T], F32R, tag=f"qp{i}", name=f"qp{i}")
                  for i in range(2)]
            if ROWTILE:
                # k in the same (hA | hB) layout; S matmuls row-tile the
                # two K=64 heads into PE row groups 0-1 / 2-3.
                kp = [apool.tile([128, T], F32R, tag=f"kp{i}", name=f"kp{i}")
                      for i in range(2)]
            else:
                # k zero-padded per head so S matmuls present K=128.
                kz = [apool.tile([128, T], F32R, tag=f"kz{i}", name=f"kz{i}")
                      for i in range(4)]
            vaug = apool.tile([128, NJ, HPC * (HD + 1)], F32R)

            # ---- QKV projection ----
            # q/k in transposed layout: psum[c_out_pair, t] = W^T x^T
            with tc.tile_pool(name="xtp", bufs=1) as xtp, \
                 tc.tile_pool(name="pproj", bufs=4, space="PSUM") as pp:
                xt = xtp.tile([128, CI, T], F32R)      # x^T, c_in chunked
                wqk = xtp.tile([128, CI, 512], F32R)
                wv = xtp.tile([128, CI, HPC * HD], F32R)
                qt4 = T // 4
                # DMA order: first-needed first. co=0 walks the four
                # T-quarters in order, so interleave wqk with quarter 0,
                # then stream the later quarters / wv / wo behind it.
                for ci in range(CI):
                    nc.sync.dma_start(wqk[:, ci, :],
                                      wqk_d[ci * 128:(ci + 1) * 128, :])
                    nc.sync.dma_start(xt[:, ci, 0:qt4],
                                      xt_d[ci * 128:(ci + 1) * 128, 0:qt4])
                for ci in range(CI):
                    nc.sync.dma_start(xt[:, ci, qt4:2 * qt4],
                                      xt_d[ci * 128:(ci + 1) * 128,
                                           qt4:2 * qt4])
                for ci in range(CI):
                    nc.sync.dma_start(xt[:, ci, 2 * qt4:3 * qt4],
                                      xt_d[ci * 128:(ci + 1) * 128,
                                           2 * qt4:3 * qt4])
                for ci in range(CI):
                    nc.sync.dma_start(wv[:, ci, :],
                                      wv_d[ci * 128:(ci + 1) * 128, :])
                    nc.sync.dma_start(xt[:, ci, 3 * qt4:T],
                                      xt_d[ci * 128:(ci + 1) * 128, 3 * qt4:T])
                nc.sync.dma_start(wo[:, 0, :], wo_d[0:128, :])
                nc.sync.dma_start(wo[:, 1, :], wo_d[128:256, :])
                for t in range(n_tiles):
                    nc.sync.dma_start(masks[:, t * QB:(t + 1) * QB], mask_d[t])
                if apply_qk_bias:
                    nc.sync.dma_start(bqk[:], bqk_d)
                if apply_v_bias:
                    nc.sync.dma_start(bv[:], bv_d)
                if not ROWTILE:
                    for p in range(2):
                        nc.vector.tensor_copy(
                            kz[2 * p][64:128, :],
                            nc.const_aps.tensor(0.0, (64, T)))
                        nc.vector.tensor_copy(
                            kz[2 * p + 1][0:64, :],
                            nc.const_aps.tensor(0.0, (64, T)))
                va = vaug[:].rearrange("p j (h d) -> p j h d", h=HPC)
                nc.vector.tensor_copy(
                    va[:, :, :, HD:HD + 1],
                    nc.const_aps.tensor(1.0, (128, NJ, HPC, 1)))
                for co in (0, 1, "v", 2, 3):
                    if co == "v":
                        for tj in range(NJ):
                            psv = pp.tile([128, HPC * HD], F32, tag="pv",
                                          name="psv")
                            for ci in range(CI):
                                nc.tensor.matmul(
                                    psv[:],
                                    xt[:, ci, tj * 128:(tj + 1) * 128],
                                    wv[:, ci, :],
                                    start=(ci == 0), stop=(ci == CI - 1))
                            nc.vector.tensor_copy(
                                va[:, tj, :, 0:HD],
                                psv[:].rearrange("p (h d) -> p h d", h=HPC))
                        continue
                    pair, is_k = co // 2, co % 2
                    for ts in range(T // 512):
                        sl = slice(ts * 512, (ts + 1) * 512)
                        ps = pp.tile([128, 512], F32, tag="pqk")
                        for ci in range(CI):
                            nc.tensor.matmul(
                                ps[:],
                                wqk[:, ci, co * 128:(co + 1) * 128],
                                xt[:, ci, sl],
                                start=(ci == 0), stop=(ci == CI - 1))
                        if is_k:
                            if ROWTILE:
                                dsts = [(kp[pair][:, sl], ps[:],
                                         bqk[:, co:co + 1])]
                            else:
                                dsts = [(kz[2 * pair][0:64, sl], ps[0:64, :],
                                         bqk[0:64, co:co + 1]),
                                        (kz[2 * pair + 1][64:128, sl],
                                         ps[64:128, :],
                                         bqk[64:128, co:co + 1])]
                        else:
                            dsts = [(qp[pair][:, sl], ps[:],
                                     bqk[:, co:co + 1])]
                        for dst_ap, src_ap, b_ap in dsts:
                            if apply_qk_bias:
                                nc.scalar.activation(dst_ap, src_ap,
                                                     AF.Identity, bias=b_ap,
                                                     scale=1.0)
                            else:
                                nc.vector.tensor_copy(dst_ap, src_ap)

            # ---- attention (S^T layout) + interleaved output projection ----
            with tc.tile_pool(name="attnp", bufs=1) as attnp:
              attn = [attnp.tile([128, T], F32R, tag=f"attn{p}",
                                 name=f"attn{p}") for p in range(2)]
              with tc.tile_pool(name="psattn", bufs=1, space="PSUM") as sp, \
                   tc.tile_pool(name="psout", bufs=1, space="PSUM") as op, \
                   tc.tile_pool(name="ptp", bufs=5) as ptp, \
                   tc.tile_pool(name="divp", bufs=2) as divp, \
                   tc.tile_pool(name="osb", bufs=2) as osb:
                  pending_div = []   # (pair, [out_ps hl0, hl1], s)
                  pending_proj = []  # token-block indices ready to project

                  def emit_division(pair, tiles_hl, s):
                      # per-slab softmax divide: reciprocal of the PV
                      # denominator row, broadcast across partitions, and
                      # multiply fused into the PSUM->SBUF copy.
                      recs = []
                      for hl in range(2):
                          sums = divp.tile([1, SLAB], F32, tag="sums",
                                           name="sums")
                          nc.vector.tensor_copy(sums[:],
                                                tiles_hl[hl][HD:HD + 1, :])
                          rec1 = divp.tile([1, SLAB], F32, tag="rec1",
                                           name="rec1")
                          nc.vector.reciprocal_approx_fast(rec1[:], sums[:])
                          rec128 = divp.tile([128, SLAB], F32,
                                             tag=f"rec128_{hl}",
                                             name="rec128")
                          nc.gpsimd.partition_broadcast(rec128[:], rec1[:])
                          recs.append(rec128)
                      for hl in range(2):
                          dst = attn[pair][64 * hl:64 * hl + 64,
                                           s * SLAB:(s + 1) * SLAB]
                          nc.vector.tensor_tensor(
                              out=dst, in0=tiles_hl[hl][0:HD, :],
                              in1=recs[hl][64 * hl:64 * hl + 64, :], op=MULT)
                          if apply_v_bias:
                              nc.vector.tensor_scalar(
                                  out=dst, in0=dst,
                                  scalar1=bv[64 * hl:64 * hl + 64,
                                             pair:pair + 1],
                                  scalar2=None, op0=mybir.AluOpType.add)
                      if pair == 1:
                          pending_proj.extend(range(BPS * s, BPS * (s + 1)))

                  def emit_outproj(ts):
                      # one 128-token block of the output projection; shares
                      # the S-matmul PSUM ring ("sst").
                      ps = sp.tile([128, 2, SLAB], F32, tag="sst",
                                   name="psop", bufs=2)
                      for h in range(2):
                          for pair in range(2):
                              nc.tensor.matmul(
                                  ps[:, h, :],
                                  attn[pair][:, ts * 128:(ts + 1) * 128],
                                  wo[:, pair, h * 512:(h + 1) * 512],
                                  start=(pair == 0), stop=(pair == 1))
                      ot = osb.tile([128, C], F32, tag="ot", name="ot")
                      nc.vector.tensor_copy(
                          ot[:].rearrange("p (h q) -> p h q", h=2), ps[:])
                      nc.sync.dma_start(out_d[ts * 128:(ts + 1) * 128, :],
                                        ot[:])

                  for pair in range(2):
                      q_t = qp[pair]
                      for s in range(NSLAB):
                          i_lo, i_hi = s * BPS, (s + 1) * BPS
                          chunks = []
                          for j in range(NJ):
                              live = [i for i in range(i_lo, i_hi)
                                      if btype[j, i] != 1]
                              if live:
                                  chunks.append((j, min(live), max(live)))
                          out_ps = [op.tile([HD + 1, SLAB], F32,
                                            tag=f"outps{_hl}",
                                            name=f"outps{_hl}", bufs=2)
                                    for _hl in range(2)]
                          written = np.zeros(BPS, dtype=bool)
                          for cn, (j, i0, i1) in enumerate(chunks):
                              n_cols = (i1 - i0 + 1) * QB
                              r0 = i0 - i_lo
                              # S^T for both heads into the two banks of one
                              # psum tile; one exp covers both
                              sps = sp.tile([128, 2, SLAB], F32,
                                            tag="sst", name="sst", bufs=2)
                              if ROWTILE:
                                  for hl in range(2):
                                      nc.tensor.matmul(
                                          sps[:, hl, 0:n_cols],
                                          kp[pair][64 * hl:64 * hl + 64,
                                                   j * KB:(j + 1) * KB],
                                          q_t[64 * hl:64 * hl + 64,
                                              i0 * QB:i0 * QB + n_cols],
                                          start=True, stop=True)
                              else:
                                  for hl in range(2):
                                      nc.tensor.matmul(
                                          sps[:, hl, 0:n_cols],
                                          kz[2 * pair + hl][:,
                                                            j * KB:(j + 1) * KB],
                                          q_t[:, i0 * QB:i0 * QB + n_cols],
                                          start=True, stop=True)
                              pt = ptp.tile([128, 2, SLAB], F32R, tag="pt",
                                            name="pt")
                              nc.scalar.activation(pt[:, :, 0:n_cols],
                                                   sps[:, :, 0:n_cols],
                                                   AF.Exp, scale=SCALE)
                              # masks run on DVE so gpsimd only ever runs
                              # PartitionBroadcast (no Q7 library reloads)
                              for i in range(i0, i1 + 1):
                                  rel = (i - i0) * QB
                                  if btype[j, i] == 2:
                                      ti = tidx[j, i]
                                      m2 = masks[:, ti * QB:(ti + 1) * QB] \
                                          .unsqueeze(1).broadcast_to(
                                              [128, 2, QB])
                                      nc.vector.tensor_tensor(
                                          out=pt[:, :, rel:rel + QB],
                                          in0=pt[:, :, rel:rel + QB],
                                          in1=m2, op=MULT)
                                  elif btype[j, i] == 1:
                                      nc.vector.tensor_copy(
                                          pt[:, :, rel:rel + QB],
                                          nc.const_aps.tensor(
                                              0.0, (128, 2, QB)))
                              # previous slab's division goes out right after
                              # this slab's first chunk is in flight
                              if cn == 1 and pending_div:
                                  emit_division(*pending_div.pop(0))
                              # outproj insertions go in pairs so the shared
                              # "sst" psum ring keeps its even double-buffer
                              # phase for the S-matmul pipeline
                              if cn >= 2 and cn % 2 == 0 and pending_proj:
                                  emit_outproj(pending_proj.pop(0))
                                  if pending_proj:
                                      emit_outproj(pending_proj.pop(0))
                              # PV accumulation (runs are <= 512 so no bank
                              # crossing; split only on first-write transitions)
                              segs = []
                              c = r0 * QB
                              end = (i1 - i_lo + 1) * QB
                              while c < end:
                                  st = written[c // QB]
                                  cc = c + QB
                                  while cc < end and written[cc // QB] == st:
                                      cc += QB
                                  segs.append((c, cc, not st))
                                  c = cc
                              last = cn == len(chunks) - 1
                              for hl in range(2):
                                  hh = 2 * pair + hl
                                  for (c0, c1, st_flag) in segs:
                                      nc.tensor.matmul(
                                          out_ps[hl][:, c0:c1],
                                          vaug[:, j, hh * (HD + 1):
                                               (hh + 1) * (HD + 1)],
                                          pt[:, hl, c0 - r0 * QB:c1 - r0 * QB],
                                          start=st_flag, stop=last,
                                          skip_group_check=True)
                              for rr in range(r0, i1 - i_lo + 1):
                                  written[rr] = True
                          pending_div.append((pair, out_ps, s))
                  while pending_div:
                      emit_division(*pending_div.pop(0))
                  while pending_proj:
                      emit_outproj(pending_proj.pop(0))

    nc.compile()
    return nc


def _get_program(mask_bool, apply_qk_bias, apply_v_bias):
    key = (mask_bool.tobytes(), apply_qk_bias, apply_v_bias, ROWTILE)
    if key not in _cache:
        btype, tidx, tiles = _build_plan(mask_bool)
        nc = _build_program(btype, tidx, len(tiles), apply_qk_bias,
                            apply_v_bias)
        _cache[key] = (nc, tiles)
    return _cache[key]


def kernel(x, attention_mask, Wqkv, bqkv, Wo, bo, _trace=False):
    from concourse.bass_utils import run_bass_kernel_spmd

    x = np.asarray(x, dtype=np.float32)
    mask_bool = np.asarray(attention_mask)[0, 0] != 0
    Wqkv = np.asarray(Wqkv, dtype=np.float32)
    bqkv = np.asarray(bqkv, dtype=np.float32)
    Wo = np.asarray(Wo, dtype=np.float32)
    bo = np.asarray(bo, dtype=np.float32)

    apply_qk_bias = bool(np.any(bqkv[:2 * C]))
    apply_v_bias = bool(np.any(bqkv[2 * C:]))
    nc, tiles = _get_program(mask_bool, apply_qk_bias, apply_v_bias)

    xts = [_round_fp32r(x[b].T) for b in range(B)]
    in_maps = []
    for c in range(NCORES):
        b, g = divmod(c, NCORES // B)
        hs = [HPC * g + i for i in range(HPC)]
        # wqk column chunks: [q_h0|q_h1, k_h0|k_h1, q_h2|q_h3, k_h2|k_h3]
        cols, bias_cols = [], []
        for pair in range(2):
            ha, hb = hs[2 * pair], hs[2 * pair + 1]
            for base in (0, C):  # q then k offset in Wqkv columns
                cols.append(Wqkv[:, base + ha * HD:base + (ha + 1) * HD])
                cols.append(Wqkv[:, base + hb * HD:base + (hb + 1) * HD])
                bias_cols.append(np.concatenate([
                    bqkv[base + ha * HD:base + (ha + 1) * HD],
                    bqkv[base + hb * HD:base + (hb + 1) * HD]]))
        wqk_c = _round_fp32r(np.concatenate(cols, axis=1))
        bqk_c = np.stack(bias_cols, axis=1).astype(np.float32)
        wv_c = _round_fp32r(np.concatenate(
            [Wqkv[:, 2 * C + h * HD:2 * C + (h + 1) * HD] for h in hs], axis=1))
        wo_c = _round_fp32r(np.concatenate(
            [Wo[h * HD:(h + 1) * HD, :] for h in hs], axis=0))
        bv_c = np.zeros((128, 2), dtype=np.float32)
        for pair in range(2):
            ha, hb = hs[2 * pair], hs[2 * pair + 1]
            bv_c[0:HD, pair] = bqkv[2 * C + ha * HD:2 * C + (ha + 1) * HD]
            bv_c[HD:128, pair] = bqkv[2 * C + hb * HD:2 * C + (hb + 1) * HD]
        in_maps.append({
            "xt": xts[b], "wqk": wqk_c, "wv": wv_c, "wo": wo_c,
            "masks": tiles, "bqk": bqk_c, "bv": bv_c,
        })

    kwargs = {}
    if _trace:
        kwargs = dict(trace=True, trace_cores=[0])
    res = run_bass_kernel_spmd(nc, in_maps, core_ids=list(range(NCORES)),
                               **kwargs)
    out = np.empty((B, T, C), dtype=np.float32)
    gpb = NCORES // B
    for b in range(B):
        acc = res.results[b * gpb]["out"].astype(np.float32)
        for g in range(1, gpb):
            acc = acc + res.results[b * gpb + g]["out"]
        out[b] = acc + bo
    if _trace:
        kernel._last_results = res
    return out


# revision 13
# speedup vs baseline: 1.4500x; 1.0705x over previous
"""Multi-head self-attention (B=2, T=2048, C=1024, H=16) on 8 trn2 cores.

Sharding: core c -> batch b = c//4, heads 4*(c%4) .. 4*(c%4)+3.
Each core: QKV projection for its 4 heads, causal attention in S^T layout
(keys on partitions), partial output projection over its heads' rows of Wo.
Host sums the 4 partials per batch element and adds bo.

Structure (v2):
- softmax division is per-slab: reciprocal of the PV denominator row,
  gpsimd partition-broadcast, and the divide fused into the PSUM->SBUF
  copy of the PV output. Division for slab s is emitted inside slab s+1
  so it never blocks the attention pipeline.
- output projection is interleaved into the attention loop (one token
  block at a time, as soon as both head-pairs of its slab are divided),
  sharing the S-matmul PSUM ring; its DMA drains during attention.
- ROWTILE: S matmuls use PE row tiling (two K=64 heads run concurrently
  in row groups 0-1 / 2-3) instead of zero-padding K to 128.

All matmuls run in float32r (fp32 with 12-bit mantissa, full PE rate).
"""
import sys

sys.path.insert(0, "/opt/trn_rl_repo")

import numpy as np

B, T, C, H = 2, 2048, 1024, 16
HD = C // H            # 64
NCORES = 8
HPC = H // (NCORES // B)   # heads per core = 4
QB = 128               # q block (columns of S^T)
KB = 128               # k chunk (partitions of S^T)
NJ = T // KB           # 16
NI = T // QB           # 16
SLAB = 512             # q columns processed per attention pass
NSLAB = T // SLAB      # 4
BPS = SLAB // QB       # q blocks per slab = 4
CI = C // 128          # 8 contraction chunks for projections
SCALE = HD ** -0.5
ROWTILE = True

_cache = {}


def _round_fp32r(x: np.ndarray) -> np.ndarray:
    u = np.ascontiguousarray(x, dtype=np.float32).view(np.uint32)
    r = (u + 0x7FF + ((u >> 12) & 1)) & np.uint32(0xFFFFF000)
    return r.view(np.float32)


def _build_plan(mask_bool: np.ndarray):
    """mask_bool: [T, T] (q, k). Returns per (j, i) block types and tiles.

    type 0 = all valid (no mask work), 1 = all masked (skip), 2 = mixed.
    Tiles are stored transposed to match S^T ([k_local, q_local])."""
    btype = np.zeros((NJ, NI), dtype=np.int32)
    tidx = np.full((NJ, NI), -1, dtype=np.int32)
    tiles = []
    tile_map = {}
    for j in range(NJ):
        for i in range(NI):
            sub = mask_bool[i * QB:(i + 1) * QB, j * KB:(j + 1) * KB]
            if sub.all():
                btype[j, i] = 0
            elif not sub.any():
                btype[j, i] = 1
            else:
                btype[j, i] = 2
                key = sub.tobytes()
                if key not in tile_map:
                    tile_map[key] = len(tiles)
                    tiles.append(sub.T.astype(np.float32))
                tidx[j, i] = tile_map[key]
    if not tiles:
        tiles.append(np.ones((KB, QB), dtype=np.float32))
    return btype, tidx, np.stack(tiles)


def _build_program(btype, tidx, n_tiles, apply_qk_bias, apply_v_bias):
    import concourse.bass as bass
    import concourse.tile as tile
    import concourse.mybir as mybir
    from concourse import bacc

    F32 = mybir.dt.float32
    F32R = mybir.dt.float32r
    AF = mybir.ActivationFunctionType
    MULT = mybir.AluOpType.mult

    nc = bacc.Bacc("TRN2", target_bir_lowering=False, debug=False)
    xt_d = nc.dram_tensor("xt", [C, T], F32R, kind="ExternalInput").ap()
    wqk_d = nc.dram_tensor("wqk", [C, 4 * 128], F32R, kind="ExternalInput").ap()
    wv_d = nc.dram_tensor("wv", [C, HPC * HD], F32R, kind="ExternalInput").ap()
    wo_d = nc.dram_tensor("wo", [HPC * HD, C], F32R, kind="ExternalInput").ap()
    mask_d = nc.dram_tensor("masks", [n_tiles, KB, QB], F32,
                            kind="ExternalInput").ap()
    bqk_d = nc.dram_tensor("bqk", [128, 4], F32, kind="ExternalInput").ap()
    bv_d = nc.dram_tensor("bv", [128, 2], F32, kind="ExternalInput").ap()
    out_d = nc.dram_tensor("out", [T, C], F32, kind="ExternalOutput").ap()

    with tile.TileContext(nc) as tc:
        with tc.tile_pool(name="weights", bufs=1) as wpool, \
             tc.tile_pool(name="acts", bufs=1) as apool:
            # ---- resident SBUF tensors ----
            wo = wpool.tile([128, 2, C], F32R)
            masks = wpool.tile([128, n_tiles * QB], F32)
            bqk = wpool.tile([128, 4], F32)
            bv = wpool.tile([128, 2], F32)
            # q tiles hold (q_hA | q_hB) on partitions 0-63 / 64-127.
            qp = [apool.tile([128, T], F32R, tag=f"qp{i}", name=f"qp{i}")
                  for i in range(2)]
            if ROWTILE:
                # k in the same (hA | hB) layout; S matmuls row-tile the
                # two K=64 heads into PE row groups 0-1 / 2-3.
                kp = [apool.tile([128, T], F32R, tag=f"kp{i}", name=f"kp{i}")
                      for i in range(2)]
            else:
                # k zero-padded per head so S matmuls present K=128.
                kz = [apool.tile([128, T], F32R, tag=f"kz{i}", name=f"kz{i}")
                      for i in range(4)]
            vaug = apool.tile([128, NJ, HPC * (HD + 1)], F32R)

            # ---- fused projection + attention + output projection ----
            # The QKV projection for token slab s is emitted immediately
            # before the attention slab s, so the PE sees one dense
            # instruction stream (keeps the HAM clock gate at 8/8).
            # All PSUM chains (projection, S, outproj) share the "sst" ring.
            with tc.tile_pool(name="xtp", bufs=1) as xtp, \
                 tc.tile_pool(name="attnp", bufs=1) as attnp:
              xt = xtp.tile([128, CI, T], F32R)      # x^T, c_in chunked
                # dense_transformer note: wqk columns grouped per head pair
              wqk = xtp.tile([128, CI, 512], F32R)
              wv = xtp.tile([128, CI, HPC * HD], F32R)
              attn = [attnp.tile([128, T], F32R, tag=f"attn{p}",
                                 name=f"attn{p}") for p in range(2)]
              qt4 = T // 4
              # DMA order: first-needed first. The slab-0 projection needs
              # wqk + the first T-quarter of x^T; stream the rest behind.
              for ci in range(CI):
                  nc.sync.dma_start(wqk[:, ci, :],
                                    wqk_d[ci * 128:(ci + 1) * 128, :])
                  nc.sync.dma_start(xt[:, ci, 0:qt4],
                                    xt_d[ci * 128:(ci + 1) * 128, 0:qt4])
              for ci in range(CI):
                  nc.sync.dma_start(wv[:, ci, :],
                                    wv_d[ci * 128:(ci + 1) * 128, :])
              for ci in range(CI):
                  nc.sync.dma_start(xt[:, ci, qt4:2 * qt4],
                                    xt_d[ci * 128:(ci + 1) * 128,
                                         qt4:2 * qt4])
              nc.sync.dma_start(wo[:, 0, :], wo_d[0:128, :])
              nc.sync.dma_start(wo[:, 1, :], wo_d[128:256, :])
              for ci in range(CI):
                  nc.sync.dma_start(xt[:, ci, 2 * qt4:3 * qt4],
                                    xt_d[ci * 128:(ci + 1) * 128,
                                         2 * qt4:3 * qt4])
              for ci in range(CI):
                  nc.sync.dma_start(xt[:, ci, 3 * qt4:T],
                                    xt_d[ci * 128:(ci + 1) * 128, 3 * qt4:T])
              for t in range(n_tiles):
                  nc.sync.dma_start(masks[:, t * QB:(t + 1) * QB], mask_d[t])
              if apply_qk_bias:
                  nc.sync.dma_start(bqk[:], bqk_d)
              if apply_v_bias:
                  nc.sync.dma_start(bv[:], bv_d)
              if not ROWTILE:
                  for p in range(2):
                      nc.vector.tensor_copy(
                          kz[2 * p][64:128, :],
                          nc.const_aps.tensor(0.0, (64, T)))
                      nc.vector.tensor_copy(
                          kz[2 * p + 1][0:64, :],
                          nc.const_aps.tensor(0.0, (64, T)))
              va = vaug[:].rearrange("p j (h d) -> p j h d", h=HPC)
              nc.vector.tensor_copy(
                  va[:, :, :, HD:HD + 1],
                  nc.const_aps.tensor(1.0, (128, NJ, HPC, 1)))
              with tc.tile_pool(name="psattn", bufs=1, space="PSUM") as sp, \
                   tc.tile_pool(name="psout", bufs=1, space="PSUM") as op, \
                   tc.tile_pool(name="ptp", bufs=4) as ptp, \
                   tc.tile_pool(name="divp", bufs=2) as divp, \
                   tc.tile_pool(name="osb", bufs=2) as osb:
                  pending_div = []   # (pair, [out_ps hl0, hl1], s)
                  pending_proj = []  # token-block indices ready to project

                  def emit_division(pair, tiles_hl, s):
                      # per-slab softmax divide: reciprocal of the PV
                      # denominator row, broadcast across partitions, and
                      # multiply fused into the PSUM->SBUF copy.
                      recs = []
                      for hl in range(2):
                          sums = divp.tile([1, SLAB], F32, tag="sums",
                                           name="sums")
                          nc.vector.tensor_copy(sums[:],
                                                tiles_hl[hl][HD:HD + 1, :])
                          rec1 = divp.tile([1, SLAB], F32, tag="rec1",
                                           name="rec1")
                          nc.vector.reciprocal_approx_fast(rec1[:], sums[:])
                          rec128 = divp.tile([128, SLAB], F32,
                                             tag=f"rec128_{hl}",
                                             name="rec128", bufs=1)
                          nc.gpsimd.partition_broadcast(rec128[:], rec1[:])
                          recs.append(rec128)
                      for hl in range(2):
                          dst = attn[pair][64 * hl:64 * hl + 64,
                                           s * SLAB:(s + 1) * SLAB]
                          nc.vector.tensor_tensor(
                              out=dst, in0=tiles_hl[hl][0:HD, :],
                              in1=recs[hl][64 * hl:64 * hl + 64, :], op=MULT)
                          if apply_v_bias:
                              nc.vector.tensor_scalar(
                                  out=dst, in0=dst,
                                  scalar1=bv[64 * hl:64 * hl + 64,
                                             pair:pair + 1],
                                  scalar2=None, op0=mybir.AluOpType.add)
                      if pair == 1:
                          pending_proj.extend(range(BPS * s, BPS * (s + 1)))

                  def emit_outproj(ts):
                      # one 128-token block of the output projection; shares
                      # the S-matmul PSUM ring ("sst").
                      ps = sp.tile([128, 2, SLAB], F32, tag="sst",
                                   name="psop", bufs=2)
                      for h in range(2):
                          for pair in range(2):
                              nc.tensor.matmul(
                                  ps[:, h, :],
                                  attn[pair][:, ts * 128:(ts + 1) * 128],
                                  wo[:, pair, h * 512:(h + 1) * 512],
                                  start=(pair == 0), stop=(pair == 1))
                      ot = osb.tile([128, C], F32, tag="ot", name="ot")
                      nc.vector.tensor_copy(
                          ot[:].rearrange("p (h q) -> p h q", h=2), ps[:])
                      nc.sync.dma_start(out_d[ts * 128:(ts + 1) * 128, :],
                                        ot[:])

                  def emit_proj_slab(s):
                      # QKV projection for token slab s (q/k for both pairs
                      # plus this slab's v blocks), through the sst ring.
                      # 8 ring insertions per slab keep the phase even.
                      sl = slice(s * 512, (s + 1) * 512)
                      for co in range(4):
                          pair, is_k = co // 2, co % 2
                          ps = sp.tile([128, 2, SLAB], F32, tag="sst",
                                       name="psqk", bufs=2)
                          for ci in range(CI):
                              nc.tensor.matmul(
                                  ps[:, 0, :],
                                  wqk[:, ci, co * 128:(co + 1) * 128],
                                  xt[:, ci, sl],
                                  start=(ci == 0), stop=(ci == CI - 1))
                          if is_k:
                              if ROWTILE:
                                  dsts = [(kp[pair][:, sl], ps[:, 0, :],
                                           bqk[:, co:co + 1])]
                              else:
                                  dsts = [(kz[2 * pair][0:64, sl],
                                           ps[0:64, 0, :],
                                           bqk[0:64, co:co + 1]),
                                          (kz[2 * pair + 1][64:128, sl],
                                           ps[64:128, 0, :],
                                           bqk[64:128, co:co + 1])]
                          else:
                              dsts = [(qp[pair][:, sl], ps[:, 0, :],
                                       bqk[:, co:co + 1])]
                          for dst_ap, src_ap, b_ap in dsts:
                              if apply_qk_bias:
                                  nc.scalar.activation(dst_ap, src_ap,
                                                       AF.Identity,
                                                       bias=b_ap, scale=1.0)
                              else:
                                  nc.vector.tensor_copy(dst_ap, src_ap)
                      for tj in range(BPS * s, BPS * (s + 1)):
                          ps = sp.tile([128, 2, SLAB], F32, tag="sst",
                                       name="psv", bufs=2)
                          for ci in range(CI):
                              nc.tensor.matmul(
                                  ps[:, 0, 0:HPC * HD],
                                  xt[:, ci, tj * 128:(tj + 1) * 128],
                                  wv[:, ci, :],
                                  start=(ci == 0), stop=(ci == CI - 1))
                          nc.vector.tensor_copy(
                              va[:, tj, :, 0:HD],
                              ps[:, 0, 0:HPC * HD].rearrange(
                                  "p (h d) -> p h d", h=HPC))

                  for s in range(NSLAB):
                      emit_proj_slab(s)
                      for pair in range(2):
                          q_t = qp[pair]
                          i_lo, i_hi = s * BPS, (s + 1) * BPS
                          chunks = []
                          for j in range(NJ):
                              live = [i for i in range(i_lo, i_hi)
                                      if btype[j, i] != 1]
                              if live:
                                  chunks.append((j, min(live), max(live)))
                          out_ps = [op.tile([HD + 1, SLAB], F32,
                                            tag=f"outps{_hl}",
                                            name=f"outps{_hl}", bufs=2)
                                    for _hl in range(2)]
                          written = np.zeros(BPS, dtype=bool)
                          for cn, (j, i0, i1) in enumerate(chunks):
                              n_cols = (i1 - i0 + 1) * QB
                              r0 = i0 - i_lo
                              # S^T for both heads into the two banks of one
                              # psum tile; one exp covers both
                              sps = sp.tile([128, 2, SLAB], F32,
                                            tag="sst", name="sst", bufs=2)
                              if ROWTILE:
                                  for hl in range(2):
                                      nc.tensor.matmul(
                                          sps[:, hl, 0:n_cols],
                                          kp[pair][64 * hl:64 * hl + 64,
                                                   j * KB:(j + 1) * KB],
                                          q_t[64 * hl:64 * hl + 64,
                                              i0 * QB:i0 * QB + n_cols],
                                          start=True, stop=True)
                              else:
                                  for hl in range(2):
                                      nc.tensor.matmul(
                                          sps[:, hl, 0:n_cols],
                                          kz[2 * pair + hl][:,
                                                            j * KB:(j + 1) * KB],
                                          q_t[:, i0 * QB:i0 * QB + n_cols],
                                          start=True, stop=True)
                              pt = ptp.tile([128, 2, SLAB], F32R, tag="pt",
                                            name="pt")
                              nc.scalar.activation(pt[:, :, 0:n_cols],
                                                   sps[:, :, 0:n_cols],
                                                   AF.Exp, scale=SCALE)
                              # masks run on DVE so gpsimd only ever runs
                              # PartitionBroadcast (no Q7 library reloads)
                              for i in range(i0, i1 + 1):
                                  rel = (i - i0) * QB
                                  if btype[j, i] == 2:
                                      ti = tidx[j, i]
                                      m2 = masks[:, ti * QB:(ti + 1) * QB] \
                                          .unsqueeze(1).broadcast_to(
                                              [128, 2, QB])
                                      nc.vector.tensor_tensor(
                                          out=pt[:, :, rel:rel + QB],
                                          in0=pt[:, :, rel:rel + QB],
                                          in1=m2, op=MULT)
                                  elif btype[j, i] == 1:
                                      nc.vector.tensor_copy(
                                          pt[:, :, rel:rel + QB],
                                          nc.const_aps.tensor(
                                              0.0, (128, 2, QB)))
                              # previous slab's division goes out right after
                              # this slab's first chunk is in flight
                              if cn == 1 and pending_div:
                                  emit_division(*pending_div.pop(0))
                              # outproj insertions go in pairs so the shared
                              # "sst" psum ring keeps its even double-buffer
                              # phase for the S-matmul pipeline
                              if cn >= 2 and cn % 2 == 0 and pending_proj:
                                  emit_outproj(pending_proj.pop(0))
                                  if pending_proj:
                                      emit_outproj(pending_proj.pop(0))
                              # PV accumulation (runs are <= 512 so no bank
                              # crossing; split only on first-write transitions)
                              segs = []
                              c = r0 * QB
                              end = (i1 - i_lo + 1) * QB
                              while c < end:
                                  st = written[c // QB]
                                  cc = c + QB
                                  while cc < end and written[cc // QB] == st:
                                      cc += QB
                                  segs.append((c, cc, not st))
                                  c = cc
                              last = cn == len(chunks) - 1
                              for hl in range(2):
                                  hh = 2 * pair + hl
                                  for (c0, c1, st_flag) in segs:
                                      nc.tensor.matmul(
                                          out_ps[hl][:, c0:c1],
                                          vaug[:, j, hh * (HD + 1):
                                               (hh + 1) * (HD + 1)],
                                          pt[:, hl, c0 - r0 * QB:c1 - r0 * QB],
                                          start=st_flag, stop=last,
                                          skip_group_check=True)
                              for rr in range(r0, i1 - i_lo + 1):
                                  written[rr] = True
                          pending_div.append((pair, out_ps, s))
                  while pending_div:
                      emit_division(*pending_div.pop(0))
                  while pending_proj:
                      emit_outproj(pending_proj.pop(0))

    nc.compile()
    return nc


def _get_program(mask_bool, apply_qk_bias, apply_v_bias):
    key = (mask_bool.tobytes(), apply_qk_bias, apply_v_bias, ROWTILE)
    if key not in _cache:
        btype, tidx, tiles = _build_plan(mask_bool)
        nc = _build_program(btype, tidx, len(tiles), apply_qk_bias,
                            apply_v_bias)
        _cache[key] = (nc, tiles)
    return _cache[key]


def kernel(x, attention_mask, Wqkv, bqkv, Wo, bo, _trace=False):
    from concourse.bass_utils import run_bass_kernel_spmd

    x = np.asarray(x, dtype=np.float32)
    mask_bool = np.asarray(attention_mask)[0, 0] != 0
    Wqkv = np.asarray(Wqkv, dtype=np.float32)
    bqkv = np.asarray(bqkv, dtype=np.float32)
    Wo = np.asarray(Wo, dtype=np.float32)
    bo = np.asarray(bo, dtype=np.float32)

    apply_qk_bias = bool(np.any(bqkv[:2 * C]))
    apply_v_bias = bool(np.any(bqkv[2 * C:]))
    nc, tiles = _get_program(mask_bool, apply_qk_bias, apply_v_bias)

    xts = [_round_fp32r(x[b].T) for b in range(B)]
    in_maps = []
    for c in range(NCORES):
        b, g = divmod(c, NCORES // B)
        hs = [HPC * g + i for i in range(HPC)]
        # wqk column chunks: [q_h0|q_h1, k_h0|k_h1, q_h2|q_h3, k_h2|k_h3]
        cols, bias_cols = [], []
        for pair in range(2):
            ha, hb = hs[2 * pair], hs[2 * pair + 1]
            for base in (0, C):  # q then k offset in Wqkv columns
                cols.append(Wqkv[:, base + ha * HD:base + (ha + 1) * HD])
                cols.append(Wqkv[:, base + hb * HD:base + (hb + 1) * HD])
                bias_cols.append(np.concatenate([
                    bqkv[base + ha * HD:base + (ha + 1) * HD],
                    bqkv[base + hb * HD:base + (hb + 1) * HD]]))
        wqk_c = _round_fp32r(np.concatenate(cols, axis=1))
        bqk_c = np.stack(bias_cols, axis=1).astype(np.float32)
        wv_c = _round_fp32r(np.concatenate(
            [Wqkv[:, 2 * C + h * HD:2 * C + (h + 1) * HD] for h in hs], axis=1))
        wo_c = _round_fp32r(np.concatenate(
            [Wo[h * HD:(h + 1) * HD, :] for h in hs], axis=0))
        bv_c = np.zeros((128, 2), dtype=np.float32)
        for pair in range(2):
            ha, hb = hs[2 * pair], hs[2 * pair + 1]
            bv_c[0:HD, pair] = bqkv[2 * C + ha * HD:2 * C + (ha + 1) * HD]
            bv_c[HD:128, pair] = bqkv[2 * C + hb * HD:2 * C + (hb + 1) * HD]
        in_maps.append({
            "xt": xts[b], "wqk": wqk_c, "wv": wv_c, "wo": wo_c,
            "masks": tiles, "bqk": bqk_c, "bv": bv_c,
        })

    kwargs = {}
    if _trace:
        kwargs = dict(trace=True, trace_cores=[0])
    res = run_bass_kernel_spmd(nc, in_maps, core_ids=list(range(NCORES)),
                               **kwargs)
    out = np.empty((B, T, C), dtype=np.float32)
    gpb = NCORES // B
    for b in range(B):
        acc = res.results[b * gpb]["out"].astype(np.float32)
        for g in range(1, gpb):
            acc = acc + res.results[b * gpb + g]["out"]
        out[b] = acc + bo
    if _trace:
        kernel._last_results = res
    return out


# revision 17
# speedup vs baseline: 1.4854x; 1.0244x over previous
"""Multi-head self-attention (B=2, T=2048, C=1024, H=16) on 8 trn2 cores.

Sharding: core c -> batch b = c//4, heads 4*(c%4) .. 4*(c%4)+3.
Each core: QKV projection for its 4 heads, causal attention in S^T layout
(keys on partitions), partial output projection over its heads' rows of Wo.
Host sums the 4 partials per batch element and adds bo.

Structure (v2):
- softmax division is per-slab: reciprocal of the PV denominator row,
  gpsimd partition-broadcast, and the divide fused into the PSUM->SBUF
  copy of the PV output. Division for slab s is emitted inside slab s+1
  so it never blocks the attention pipeline.
- output projection is interleaved into the attention loop (one token
  block at a time, as soon as both head-pairs of its slab are divided),
  sharing the S-matmul PSUM ring; its DMA drains during attention.
- ROWTILE: S matmuls use PE row tiling (two K=64 heads run concurrently
  in row groups 0-1 / 2-3) instead of zero-padding K to 128.

All matmuls run in float32r (fp32 with 12-bit mantissa, full PE rate).
"""
import sys

sys.path.insert(0, "/opt/trn_rl_repo")

import numpy as np

B, T, C, H = 2, 2048, 1024, 16
HD = C // H            # 64
NCORES = 8
HPC = H // (NCORES // B)   # heads per core = 4
QB = 128               # q block (columns of S^T)
KB = 128               # k chunk (partitions of S^T)
NJ = T // KB           # 16
NI = T // QB           # 16
SLAB = 512             # q columns processed per attention pass
NSLAB = T // SLAB      # 4
BPS = SLAB // QB       # q blocks per slab = 4
CI = C // 128          # 8 contraction chunks for projections
SCALE = HD ** -0.5
ROWTILE = False

_cache = {}


def _round_fp32r(x: np.ndarray) -> np.ndarray:
    u = np.ascontiguousarray(x, dtype=np.float32).view(np.uint32)
    r = (u + 0x7FF + ((u >> 12) & 1)) & np.uint32(0xFFFFF000)
    return r.view(np.float32)


def _build_plan(mask_bool: np.ndarray):
    """mask_bool: [T, T] (q, k). Returns per (j, i) block types and tiles.

    type 0 = all valid (no mask work), 1 = all masked (skip), 2 = mixed.
    Tiles are stored transposed to match S^T ([k_local, q_local])."""
    btype = np.zeros((NJ, NI), dtype=np.int32)
    tidx = np.full((NJ, NI), -1, dtype=np.int32)
    tiles = []
    tile_map = {}
    for j in range(NJ):
        for i in range(NI):
            sub = mask_bool[i * QB:(i + 1) * QB, j * KB:(j + 1) * KB]
            if sub.all():
                btype[j, i] = 0
            elif not sub.any():
                btype[j, i] = 1
            else:
                btype[j, i] = 2
                key = sub.tobytes()
                if key not in tile_map:
                    tile_map[key] = len(tiles)
                    tiles.append(sub.T.astype(np.float32))
                tidx[j, i] = tile_map[key]
    if not tiles:
        tiles.append(np.ones((KB, QB), dtype=np.float32))
    return btype, tidx, np.stack(tiles)


def _build_program(btype, tidx, n_tiles, apply_qk_bias, apply_v_bias):
    import concourse.bass as bass
    import concourse.tile as tile
    import concourse.mybir as mybir
    from concourse import bacc

    F32 = mybir.dt.float32
    F32R = mybir.dt.float32r
    AF = mybir.ActivationFunctionType
    MULT = mybir.AluOpType.mult

    nc = bacc.Bacc("TRN2", target_bir_lowering=False, debug=False)
    BF16 = mybir.dt.bfloat16
    xt_d = nc.dram_tensor("xt", [C, T], BF16, kind="ExternalInput").ap()
    wqk_d = nc.dram_tensor("wqk", [C, 4 * 128], BF16, kind="ExternalInput").ap()
    wv_d = nc.dram_tensor("wv", [C, HPC * HD], BF16, kind="ExternalInput").ap()
    wo_d = nc.dram_tensor("wo", [HPC * HD, C], F32R, kind="ExternalInput").ap()
    mask_d = nc.dram_tensor("masks", [n_tiles, KB, QB], F32,
                            kind="ExternalInput").ap()
    bqk_d = nc.dram_tensor("bqk", [128, 4], F32, kind="ExternalInput").ap()
    bv_d = nc.dram_tensor("bv", [128, 2], F32, kind="ExternalInput").ap()
    out_d = nc.dram_tensor("out", [T, C], F32, kind="ExternalOutput").ap()

    with tile.TileContext(nc) as tc:
        with tc.tile_pool(name="weights", bufs=1) as wpool, \
             tc.tile_pool(name="acts", bufs=1) as apool:
            # ---- resident SBUF tensors ----
            wo = wpool.tile([128, 2, C], F32R)
            masks = wpool.tile([128, n_tiles * QB], F32)
            bqk = wpool.tile([128, 4], F32)
            bv = wpool.tile([128, 2], F32)
            # q tiles hold (q_hA | q_hB) on partitions 0-63 / 64-127.
            qp = [apool.tile([128, T], F32R, tag=f"qp{i}", name=f"qp{i}")
                  for i in range(2)]
            if ROWTILE:
                # k in the same (hA | hB) layout; S matmuls row-tile the
                # two K=64 heads into PE row groups 0-1 / 2-3.
                kp = [apool.tile([128, T], F32R, tag=f"kp{i}", name=f"kp{i}")
                      for i in range(2)]
            else:
                # k zero-padded per head so S matmuls present K=128.
                kz = [apool.tile([128, T], F32R, tag=f"kz{i}", name=f"kz{i}")
                      for i in range(4)]
            vaug = apool.tile([128, NJ, HPC * (HD + 1)], F32R)

            # ---- fused projection + attention + output projection ----
            # The QKV projection for token slab s is emitted immediately
            # before the attention slab s, so the PE sees one dense
            # instruction stream (keeps the HAM clock gate at 8/8).
            # All PSUM chains (projection, S, outproj) share the "sst" ring.
            with tc.tile_pool(name="xtp", bufs=1) as xtp, \
                 tc.tile_pool(name="attnp", bufs=1) as attnp:
              # projection inputs in bf16: same PE rate, half the DMA/SBUF
              xt = xtp.tile([128, CI, T], BF16)      # x^T, c_in chunked
              wqk = xtp.tile([128, CI, 512], BF16)
              wv = xtp.tile([128, CI, HPC * HD], BF16)
              attn = [attnp.tile([128, T], F32R, tag=f"attn{p}",
                                 name=f"attn{p}") for p in range(2)]
              qt4 = T // 4
              # DMA order: first-needed first. The slab-0 projection needs
              # wqk + the first T-quarter of x^T; stream the rest behind.
              for ci in range(CI):
                  nc.sync.dma_start(wqk[:, ci, :],
                                    wqk_d[ci * 128:(ci + 1) * 128, :])
                  nc.sync.dma_start(xt[:, ci, 0:qt4],
                                    xt_d[ci * 128:(ci + 1) * 128, 0:qt4])
              for ci in range(CI):
                  nc.sync.dma_start(wv[:, ci, :],
                                    wv_d[ci * 128:(ci + 1) * 128, :])
              for ci in range(CI):
                  nc.sync.dma_start(xt[:, ci, qt4:2 * qt4],
                                    xt_d[ci * 128:(ci + 1) * 128,
                                         qt4:2 * qt4])
              nc.sync.dma_start(wo[:, 0, :], wo_d[0:128, :])
              nc.sync.dma_start(wo[:, 1, :], wo_d[128:256, :])
              for ci in range(CI):
                  nc.sync.dma_start(xt[:, ci, 2 * qt4:3 * qt4],
                                    xt_d[ci * 128:(ci + 1) * 128,
                                         2 * qt4:3 * qt4])
              for ci in range(CI):
                  nc.sync.dma_start(xt[:, ci, 3 * qt4:T],
                                    xt_d[ci * 128:(ci + 1) * 128, 3 * qt4:T])
              for t in range(n_tiles):
                  nc.sync.dma_start(masks[:, t * QB:(t + 1) * QB], mask_d[t])
              if apply_qk_bias:
                  nc.sync.dma_start(bqk[:], bqk_d)
              if apply_v_bias:
                  nc.sync.dma_start(bv[:], bv_d)
              if not ROWTILE:
                  for p in range(2):
                      nc.vector.tensor_copy(
                          kz[2 * p][64:128, :],
                          nc.const_aps.tensor(0.0, (64, T)))
                      nc.vector.tensor_copy(
                          kz[2 * p + 1][0:64, :],
                          nc.const_aps.tensor(0.0, (64, T)))
              va = vaug[:].rearrange("p j (h d) -> p j h d", h=HPC)
              nc.vector.tensor_copy(
                  va[:, :, :, HD:HD + 1],
                  nc.const_aps.tensor(1.0, (128, NJ, HPC, 1)))
              # load the gpsimd PartitionBroadcast library while the PE is
              # still waiting on input DMAs (off the critical path)
              warm = apool.tile([128, 16], F32, tag="warm", name="warm")
              nc.gpsimd.partition_broadcast(
                  warm[:], nc.const_aps.tensor(1.0, (1, 16)))
              with tc.tile_pool(name="psattn", bufs=1, space="PSUM") as sp, \
                   tc.tile_pool(name="psout", bufs=1, space="PSUM") as op, \
                   tc.tile_pool(name="ptp", bufs=4) as ptp, \
                   tc.tile_pool(name="divp", bufs=2) as divp, \
                   tc.tile_pool(name="osb", bufs=2) as osb:
                  pending_div = []   # (pair, [out_ps hl0, hl1], s)
                  pending_proj = []  # token-block indices ready to project

                  def emit_division(pair, tiles_hl, s):
                      # per-slab softmax divide: reciprocal of the PV
                      # denominator row, broadcast across partitions, and
                      # multiply fused into the PSUM->SBUF copy.
                      recs = []
                      for hl in range(2):
                          sums = divp.tile([1, SLAB], F32, tag="sums",
                                           name="sums")
                          nc.vector.tensor_copy(sums[:],
                                                tiles_hl[hl][HD:HD + 1, :])
                          rec1 = divp.tile([1, SLAB], F32, tag="rec1",
                                           name="rec1")
                          nc.vector.reciprocal_approx_fast(rec1[:], sums[:])
                          rec128 = divp.tile([128, SLAB], F32,
                                             tag=f"rec128_{hl}",
                                             name="rec128", bufs=1)
                          nc.gpsimd.partition_broadcast(rec128[:], rec1[:])
                          recs.append(rec128)
                      for hl in range(2):
                          dst = attn[pair][64 * hl:64 * hl + 64,
                                           s * SLAB:(s + 1) * SLAB]
                          nc.vector.tensor_tensor(
                              out=dst, in0=tiles_hl[hl][0:HD, :],
                              in1=recs[hl][64 * hl:64 * hl + 64, :], op=MULT)
                          if apply_v_bias:
                              nc.vector.tensor_scalar(
                                  out=dst, in0=dst,
                                  scalar1=bv[64 * hl:64 * hl + 64,
                                             pair:pair + 1],
                                  scalar2=None, op0=mybir.AluOpType.add)
                      if pair == 1:
                          pending_proj.extend(range(BPS * s, BPS * (s + 1)))

                  def emit_outproj(ts):
                      # one 128-token block of the output projection; shares
                      # the S-matmul PSUM ring ("sst").
                      ps = sp.tile([128, 2, SLAB], F32, tag="sst",
                                   name="psop", bufs=2)
                      for h in range(2):
                          for pair in range(2):
                              nc.tensor.matmul(
                                  ps[:, h, :],
                                  attn[pair][:, ts * 128:(ts + 1) * 128],
                                  wo[:, pair, h * 512:(h + 1) * 512],
                                  start=(pair == 0), stop=(pair == 1))
                      ot = osb.tile([128, C], F32, tag="ot", name="ot")
                      nc.vector.tensor_copy(
                          ot[:].rearrange("p (h q) -> p h q", h=2), ps[:])
                      nc.sync.dma_start(out_d[ts * 128:(ts + 1) * 128, :],
                                        ot[:])

                  def emit_proj_slab(s):
                      # QKV projection for token slab s (q/k for both pairs
                      # plus this slab's v blocks), through the sst ring.
                      # 8 ring insertions per slab keep the phase even.
                      sl = slice(s * 512, (s + 1) * 512)
                      for co in range(4):
                          pair, is_k = co // 2, co % 2
                          ps = sp.tile([128, 2, SLAB], F32, tag="sst",
                                       name="psqk", bufs=2)
                          for ci in range(CI):
                              nc.tensor.matmul(
                                  ps[:, 0, :],
                                  wqk[:, ci, co * 128:(co + 1) * 128],
                                  xt[:, ci, sl],
                                  start=(ci == 0), stop=(ci == CI - 1))
                          if is_k:
                              if ROWTILE:
                                  dsts = [(kp[pair][:, sl], ps[:, 0, :],
                                           bqk[:, co:co + 1])]
                              else:
                                  dsts = [(kz[2 * pair][0:64, sl],
                                           ps[0:64, 0, :],
                                           bqk[0:64, co:co + 1]),
                                          (kz[2 * pair + 1][64:128, sl],
                                           ps[64:128, 0, :],
                                           bqk[64:128, co:co + 1])]
                          else:
                              dsts = [(qp[pair][:, sl], ps[:, 0, :],
                                       bqk[:, co:co + 1])]
                          for dst_ap, src_ap, b_ap in dsts:
                              if apply_qk_bias:
                                  nc.scalar.activation(dst_ap, src_ap,
                                                       AF.Identity,
                                                       bias=b_ap, scale=1.0)
                              else:
                                  nc.vector.tensor_copy(dst_ap, src_ap)
                      for tj in range(BPS * s, BPS * (s + 1)):
                          ps = sp.tile([128, 2, SLAB], F32, tag="sst",
                                       name="psv", bufs=2)
                          for ci in range(CI):
                              nc.tensor.matmul(
                                  ps[:, 0, 0:HPC * HD],
                                  xt[:, ci, tj * 128:(tj + 1) * 128],
                                  wv[:, ci, :],
                                  start=(ci == 0), stop=(ci == CI - 1))
                          nc.vector.tensor_copy(
                              va[:, tj, :, 0:HD],
                              ps[:, 0, 0:HPC * HD].rearrange(
                                  "p (h d) -> p h d", h=HPC))

                  for s in range(NSLAB):
                      emit_proj_slab(s)
                      for pair in range(2):
                          q_t = qp[pair]
                          i_lo, i_hi = s * BPS, (s + 1) * BPS
                          chunks = []
                          for j in range(NJ):
                              live = [i for i in range(i_lo, i_hi)
                                      if btype[j, i] != 1]
                              if live:
                                  chunks.append((j, min(live), max(live)))
                          out_ps = [op.tile([HD + 1, SLAB], F32,
                                            tag=f"outps{_hl}",
                                            name=f"outps{_hl}", bufs=2)
                                    for _hl in range(2)]
                          written = np.zeros(BPS, dtype=bool)
                          for cn, (j, i0, i1) in enumerate(chunks):
                              n_cols = (i1 - i0 + 1) * QB
                              r0 = i0 - i_lo
                              # S^T for both heads into the two banks of one
                              # psum tile; one exp covers both
                              sps = sp.tile([128, 2, SLAB], F32,
                                            tag="sst", name="sst", bufs=2)
                              if ROWTILE:
                                  for hl in range(2):
                                      nc.tensor.matmul(
                                          sps[:, hl, 0:n_cols],
                                          kp[pair][64 * hl:64 * hl + 64,
                                                   j * KB:(j + 1) * KB],
                                          q_t[64 * hl:64 * hl + 64,
                                              i0 * QB:i0 * QB + n_cols],
                                          start=True, stop=True)
                              else:
                                  for hl in range(2):
                                      nc.tensor.matmul(
                                          sps[:, hl, 0:n_cols],
                                          kz[2 * pair + hl][:,
                                                            j * KB:(j + 1) * KB],
                                          q_t[:, i0 * QB:i0 * QB + n_cols],
                                          start=True, stop=True)
                              pt = ptp.tile([128, 2, SLAB], F32R, tag="pt",
                                            name="pt")
                              nc.scalar.activation(pt[:, :, 0:n_cols],
                                                   sps[:, :, 0:n_cols],
                                                   AF.Exp, scale=SCALE)
                              # masks run on DVE so gpsimd only ever runs
                              # PartitionBroadcast (no Q7 library reloads)
                              for i in range(i0, i1 + 1):
                                  rel = (i - i0) * QB
                                  if btype[j, i] == 2:
                                      ti = tidx[j, i]
                                      m2 = masks[:, ti * QB:(ti + 1) * QB] \
                                          .unsqueeze(1).broadcast_to(
                                              [128, 2, QB])
                                      nc.vector.tensor_tensor(
                                          out=pt[:, :, rel:rel + QB],
                                          in0=pt[:, :, rel:rel + QB],
                                          in1=m2, op=MULT)
                                  elif btype[j, i] == 1:
                                      nc.vector.tensor_copy(
                                          pt[:, :, rel:rel + QB],
                                          nc.const_aps.tensor(
                                              0.0, (128, 2, QB)))
                              # previous slab's division goes out right after
                              # this slab's first chunk is in flight
                              if cn == 1 and pending_div:
                                  emit_division(*pending_div.pop(0))
                              # outproj insertions go in pairs so the shared
                              # "sst" psum ring keeps its even double-buffer
                              # phase for the S-matmul pipeline
                              if cn >= 2 and cn % 2 == 0 and pending_proj:
                                  emit_outproj(pending_proj.pop(0))
                                  if pending_proj:
                                      emit_outproj(pending_proj.pop(0))
                              # PV accumulation (runs are <= 512 so no bank
                              # crossing; split only on first-write transitions)
                              segs = []
                              c = r0 * QB
                              end = (i1 - i_lo + 1) * QB
                              while c < end:
                                  st = written[c // QB]
                                  cc = c + QB
                                  while cc < end and written[cc // QB] == st:
                                      cc += QB
                                  segs.append((c, cc, not st))
                                  c = cc
                              last = cn == len(chunks) - 1
                              for hl in range(2):
                                  hh = 2 * pair + hl
                                  for (c0, c1, st_flag) in segs:
                                      nc.tensor.matmul(
                                          out_ps[hl][:, c0:c1],
                                          vaug[:, j, hh * (HD + 1):
                                               (hh + 1) * (HD + 1)],
                                          pt[:, hl, c0 - r0 * QB:c1 - r0 * QB],
                                          start=st_flag, stop=last,
                                          skip_group_check=True)
                              for rr in range(r0, i1 - i_lo + 1):
                                  written[rr] = True
                          pending_div.append((pair, out_ps, s))
                  while pending_div:
                      emit_division(*pending_div.pop(0))
                  while pending_proj:
                      emit_outproj(pending_proj.pop(0))

    nc.compile()
    return nc


def _get_program(mask_bool, apply_qk_bias, apply_v_bias):
    key = (mask_bool.tobytes(), apply_qk_bias, apply_v_bias, ROWTILE)
    if key not in _cache:
        btype, tidx, tiles = _build_plan(mask_bool)
        nc = _build_program(btype, tidx, len(tiles), apply_qk_bias,
                            apply_v_bias)
        _cache[key] = (nc, tiles)
    return _cache[key]


def kernel(x, attention_mask, Wqkv, bqkv, Wo, bo, _trace=False):
    import ml_dtypes
    from concourse.bass_utils import run_bass_kernel_spmd

    bf16 = ml_dtypes.bfloat16
    x = np.asarray(x, dtype=np.float32)
    mask_bool = np.asarray(attention_mask)[0, 0] != 0
    Wqkv = np.asarray(Wqkv, dtype=np.float32)
    bqkv = np.asarray(bqkv, dtype=np.float32)
    Wo = np.asarray(Wo, dtype=np.float32)
    bo = np.asarray(bo, dtype=np.float32)

    apply_qk_bias = bool(np.any(bqkv[:2 * C]))
    apply_v_bias = bool(np.any(bqkv[2 * C:]))
    nc, tiles = _get_program(mask_bool, apply_qk_bias, apply_v_bias)

    xts = [np.ascontiguousarray(x[b].T).astype(bf16) for b in range(B)]
    in_maps = []
    for c in range(NCORES):
        b, g = divmod(c, NCORES // B)
        hs = [HPC * g + i for i in range(HPC)]
        # wqk column chunks: [q_h0|q_h1, k_h0|k_h1, q_h2|q_h3, k_h2|k_h3]
        cols, bias_cols = [], []
        for pair in range(2):
            ha, hb = hs[2 * pair], hs[2 * pair + 1]
            for base in (0, C):  # q then k offset in Wqkv columns
                cols.append(Wqkv[:, base + ha * HD:base + (ha + 1) * HD])
                cols.append(Wqkv[:, base + hb * HD:base + (hb + 1) * HD])
                bias_cols.append(np.concatenate([
                    bqkv[base + ha * HD:base + (ha + 1) * HD],
                    bqkv[base + hb * HD:base + (hb + 1) * HD]]))
        wqk_c = np.concatenate(cols, axis=1).astype(bf16)
        bqk_c = np.stack(bias_cols, axis=1).astype(np.float32)
        wv_c = np.concatenate(
            [Wqkv[:, 2 * C + h * HD:2 * C + (h + 1) * HD] for h in hs],
            axis=1).astype(bf16)
        wo_c = _round_fp32r(np.concatenate(
            [Wo[h * HD:(h + 1) * HD, :] for h in hs], axis=0))
        bv_c = np.zeros((128, 2), dtype=np.float32)
        for pair in range(2):
            ha, hb = hs[2 * pair], hs[2 * pair + 1]
            bv_c[0:HD, pair] = bqkv[2 * C + ha * HD:2 * C + (ha + 1) * HD]
            bv_c[HD:128, pair] = bqkv[2 * C + hb * HD:2 * C + (hb + 1) * HD]
        in_maps.append({
            "xt": xts[b], "wqk": wqk_c, "wv": wv_c, "wo": wo_c,
            "masks": tiles, "bqk": bqk_c, "bv": bv_c,
        })

    kwargs = {}
    if _trace:
        kwargs = dict(trace=True, trace_cores=[0])
    res = run_bass_kernel_spmd(nc, in_maps, core_ids=list(range(NCORES)),
                               **kwargs)
    out = np.empty((B, T, C), dtype=np.float32)
    gpb = NCORES // B
    for b in range(B):
        acc = res.results[b * gpb]["out"].astype(np.float32)
        for g in range(1, gpb):
            acc = acc + res.results[b * gpb + g]["out"]
        out[b] = acc + bo
    if _trace:
        kernel._last_results = res
    return out
